# revision 15
# baseline (speedup 1.0000x reference)
"""Trainium2 Bass kernel for CustomizablePatchDominantGradientOrientation.

Pipeline per patch (32x32, fp32):
  sobel (replicate pad, [1,2,1]x[-1,0,1] separable; /8 dropped - the final
  angle is invariant to a global scale on (gx, gy, mag)) -> on GPSIMD
  mag = sqrt(gx^2+gy^2+eps) on ACT (feeds only tent weights; table accuracy
  is ample), theta = atan2(gy, gx) computed directly: DVE clamp+recip, GPSIMD
  multiply, ACT arctan, DVE quadrant-select custom op.
  soft 36-bin histogram of theta weighted by mag: per bin ONE custom-DVE
  running-sum op (scan body) over a [128, 4, 1024] 4-tile supergroup with a
  stride-0-collapsed output -> per-column cumsums; per-patch-column bins
  recovered by adjacent diffs.  circular smoothing, argmax, parabolic
  refinement -> angle (tail identical to baseline).

Data parallel: B=32768 patches sharded over 8 NeuronCores (4096 each);
per core 32 tiles of [128 patches x 1024 pixels], grouped 4 tiles per
ACT-table-set phase group = one histogram supergroup.
"""

import math

import numpy as np

NBINS = 36
PI = math.pi
PATCH = 32
HW = PATCH * PATCH
P = 128          # partitions (patches per tile)
N_CORES = 8
GROUP = 4        # tiles per phase group == histogram supergroup width

_BUILD_CACHE = {}
_OPS_REGISTERED = {}


# --------------------------------------------------------------------------
# custom DVE ops
# --------------------------------------------------------------------------
def _register_custom_ops():
    """Register the fused ops at runtime (row assignment + sha pin, exactly
    what a source-level `OPS.append` would do)."""
    if _OPS_REGISTERED:
        return _OPS_REGISTERED

    import concourse.dve_ops as dve_ops
    from concourse.dve_ops import DveOp
    from concourse.dve_spec import (
        Spec, Src0, Src1, C0, C1, Zero, relu, minn, maxx, select, scan,
        AluOp, lower, _has_src1,
    )
    from concourse.dve_uop import DveOpSpec

    def _reg(name, spec):
        if name in dve_ops._SUB_OPCODE_FOR_NAME:
            for op in dve_ops.OPS:
                if op.name == name:
                    return op
        row = dve_ops._CUSTOM_DVE_ROW_BASE + len(dve_ops.OPS)
        assert row < 0x20, "custom-DVE row budget exhausted"
        dve_ops._SUB_OPCODE_FOR_NAME[name] = row
        shas = {}
        for ver in ("v3", "v4"):
            s = DveOpSpec(name=name, opcode=row, uops=lower(spec, ver=ver),
                          rd1_en=_has_src1(spec))
            shas[ver] = s.sha(ver)
        op = DveOp(name, spec, subdim=False, uops_sha=shas)
        dve_ops.OPS.append(op)
        dve_ops.CUSTOM_DVE_SPECS[name] = spec
        return op

    def _tri_scan_ref(in0, in1, s0, s1, imm2):
        b = (np.maximum(np.minimum(in0 - s0, s1 - in0), 0.0)
             .astype(np.float32) * in1)
        sh = b.shape
        return np.cumsum(b.reshape(sh[0], -1), axis=-1).reshape(sh)

    def _wrap_scan_ref(in0, in1, s0, s1, imm2):
        b = (np.maximum(np.maximum(s0 - in0, in0 - s1), 0.0)
             .astype(np.float32) * in1)
        sh = b.shape
        return np.cumsum(b.reshape(sh[0], -1), axis=-1).reshape(sh)

    def _clampadd_ref(in0, in1, s0, s1, imm2):
        t0 = (in0 + s0).astype(np.float32)
        return np.where(t0 >= 0, np.maximum(t0, s1),
                        np.minimum(t0, -s1)).astype(np.float32)

    def _quadadd_ref(in0, in1, s0, s1, imm2):
        adj = np.where(in1 < 0, np.where(in0 >= 0, -s0, s0), 0.0)
        return (in0 + adj).astype(np.float32)

    tri = relu(minn(Src0 - C0, C1 - Src0)) * Src1
    _OPS_REGISTERED["tri_scan"] = _reg(
        "HIST_TRI_SCAN_ANT",
        Spec(body=scan(AluOp.ADD, tri), reference=_tri_scan_ref))
    wrap = relu(maxx(C0 - Src0, Src0 - C1)) * Src1
    _OPS_REGISTERED["wrap_scan"] = _reg(
        "HIST_WRAP_SCAN_ANT",
        Spec(body=scan(AluOp.ADD, wrap), reference=_wrap_scan_ref))
    t0 = Src0 + C0
    _OPS_REGISTERED["clampadd"] = _reg(
        "CLAMP_ADD_ANT",
        Spec(body=select(t0 >= Zero, maxx(t0, C1), minn(t0, Zero - C1)),
             reference=_clampadd_ref))
    # t = au + (gxs<0 ? (au>=0 ? -pi : +pi) : 0)   [in0=au, in1=gxs, s0=pi]
    # sign(gy) recovered as -sign(au) when gxs<0; au==+/-0 cases land in the
    # wrap bin with identical tent weights either way.
    _OPS_REGISTERED["quadadd"] = _reg(
        "QUAD_ADD_ANT",
        Spec(body=Src0 + select(Src1 < Zero,
                                select(Src0 >= Zero, Zero - C0, C0), Zero),
             reference=_quadadd_ref))
    return _OPS_REGISTERED


# --------------------------------------------------------------------------
# kernel build
# --------------------------------------------------------------------------
def _build(b_core, smooth_w, wk_is_ones):
    import concourse.bacc as bacc
    import concourse.mybir as mybir
    from concourse.tile import TileContext
    from concourse.bass import broadcast_tensor_aps

    ops = _register_custom_ops()
    TRI_SCAN, WRAP_SCAN = ops["tri_scan"], ops["wrap_scan"]
    CLAMPADD, QUADADD = ops["clampadd"], ops["quadadd"]

    f32 = mybir.dt.float32
    Alu = mybir.AluOpType
    Act = mybir.ActivationFunctionType

    n_tiles = b_core // P
    assert b_core % P == 0 and n_tiles % GROUP == 0
    w0, w1, w2 = (float(x) for x in smooth_w)

    nc = bacc.Bacc(None, target_bir_lowering=False, debug=False)
    patch_in = nc.dram_tensor("patch", [b_core, HW], f32, kind="ExternalInput")
    # consts: iota36 repeated n_tiles times, then (iota36 - 64) repeated
    consts_in = nc.dram_tensor("consts", [P, 2 * n_tiles * NBINS], f32,
                               kind="ExternalInput")
    wk_in = None
    if not wk_is_ones:
        wk_in = nc.dram_tensor("wk", [P, HW], f32, kind="ExternalInput")
    out_t = nc.dram_tensor("angle", [b_core], f32, kind="ExternalOutput")

    # full-angle tent constants: bin k center (k-18)*pi/18, half-width pi/18
    D = PI / 18.0

    def c_lo(k):
        return (k - 19.0) * D

    def c_hi(k):
        return (k - 17.0) * D

    with TileContext(nc) as tc:
        with tc.tile_pool(name="pool", bufs=2) as pool, \
             tc.tile_pool(name="persist", bufs=1) as pp:
            IOTA = pp.tile([P, n_tiles, NBINS], f32)
            IOTA64 = pp.tile([P, n_tiles, NBINS], f32)
            nc.sync.dma_start(IOTA[:], consts_in[:, 0:n_tiles * NBINS])
            nc.sync.dma_start(IOTA64[:], consts_in[:, n_tiles * NBINS:])
            WK = None
            if wk_in is not None:
                WK = pp.tile([P, HW], f32)
                nc.sync.dma_start(WK[:], wk_in[:])

            HEXT = pp.tile([P, n_tiles, NBINS + 2], f32)
            ANG = pp.tile([P, n_tiles], f32)

            n_groups = n_tiles // GROUP
            for g in range(n_groups):
                tiles = range(g * GROUP, (g + 1) * GROUP)
                AW = pool.tile([P, GROUP, HW], f32, tag="aw", name=f"aw{g}")
                MW = pool.tile([P, GROUP, HW], f32, tag="mw", name=f"mw{g}")
                slot = {}
                # ---- phase A: sobel (gpsimd), mag (ACT sqrt table) ----
                for t in tiles:
                    s = t % GROUP
                    X = pool.tile([P, HW], f32, tag="x", bufs=3, name=f"x{t}")
                    nc.sync.dma_start(X[:], patch_in[t * P:(t + 1) * P, :])
                    X3 = X.rearrange("p (r c) -> p r c", c=PATCH)

                    SV = pool.tile([P, HW], f32, tag="sv", name=f"sv{t}")
                    T1 = pool.tile([P, HW], f32, tag="t1", bufs=1,
                                   name=f"t1{t}")
                    # vertical [1,2,1] with replicate rows, all-TT forms
                    # (GPSIMD tensor_scalar is ~5us - avoid it entirely):
                    # T1[r] = X[r] + X[r+1];  SV[r] = T1[r-1] + T1[r]
                    nc.gpsimd.tensor_tensor(
                        T1[:, 0:992], X[:, 0:992], X[:, 32:1024], Alu.add)
                    nc.gpsimd.tensor_tensor(
                        SV[:, 32:992], T1[:, 0:960], T1[:, 32:992], Alu.add)
                    # top edge: 3*X[0]+X[1] = T1[0] + 2*X[0]
                    nc.gpsimd.tensor_tensor(
                        SV[:, 0:32], T1[:, 0:32], X[:, 0:32], Alu.add)
                    nc.gpsimd.tensor_tensor(
                        SV[:, 0:32], SV[:, 0:32], X[:, 0:32], Alu.add)
                    # bottom edge: 3*X[31]+X[30] = T1[30] + 2*X[31]
                    nc.gpsimd.tensor_tensor(
                        SV[:, 992:1024], T1[:, 960:992], X[:, 992:1024],
                        Alu.add)
                    nc.gpsimd.tensor_tensor(
                        SV[:, 992:1024], SV[:, 992:1024], X[:, 992:1024],
                        Alu.add)
                    SV3 = SV.rearrange("p (r c) -> p r c", c=PATCH)

                    GX = pool.tile([P, HW], f32, tag=f"gx{s}", bufs=1,
                                   name=f"gx{t}")
                    GX3 = GX.rearrange("p (r c) -> p r c", c=PATCH)
                    # horizontal central difference: flat contiguous interior
                    # (contention-resistant), then one combined strided op
                    # overwriting both replicate-pad edge columns.
                    nc.vector.tensor_tensor(
                        GX[:, 1:1023], SV[:, 2:1024], SV[:, 0:1022],
                        Alu.subtract)
                    nc.vector.tensor_tensor(
                        GX3[:, :, 0:32:31], SV3[:, :, 1:32:30],
                        SV3[:, :, 0:31:30], Alu.subtract)

                    SH = pool.tile([P, HW], f32, tag="sh", name=f"sh{t}")
                    SH3 = SH.rearrange("p (r c) -> p r c", c=PATCH)
                    # horizontal [1,2,1]: flat interior + combined edge fix
                    nc.vector.scalar_tensor_tensor(
                        out=SH[:, 1:1023], in0=X[:, 1:1023], scalar=2.0,
                        in1=X[:, 0:1022], op0=Alu.mult, op1=Alu.add)
                    nc.vector.tensor_tensor(
                        SH[:, 1:1023], SH[:, 1:1023], X[:, 2:1024], Alu.add)
                    nc.vector.scalar_tensor_tensor(
                        out=SH3[:, :, 0:32:31], in0=X3[:, :, 0:32:31],
                        scalar=3.0, in1=X3[:, :, 1:31:29], op0=Alu.mult,
                        op1=Alu.add)

                    GY = pool.tile([P, HW], f32, tag=f"gy{s}", bufs=1,
                                   name=f"gy{t}")
                    # vertical central difference with replicate rows
                    # (contiguous - fine on GPSIMD)
                    nc.gpsimd.tensor_tensor(
                        GY[:, 32:992], SH[:, 64:1024], SH[:, 0:960],
                        Alu.subtract)
                    nc.gpsimd.tensor_tensor(
                        GY[:, 0:32], SH[:, 32:64], SH[:, 0:32], Alu.subtract)
                    nc.gpsimd.tensor_tensor(
                        GY[:, 992:1024], SH[:, 992:1024], SH[:, 960:992],
                        Alu.subtract)

                    if WK is not None:
                        nc.gpsimd.tensor_tensor(GX[:], GX[:], WK[:], Alu.mult)
                        nc.gpsimd.tensor_tensor(GY[:], GY[:], WK[:], Alu.mult)

                    # g2 = gx^2 + gy^2 (exact fp32 mults on GPSIMD); eps goes
                    # in the ACT sqrt bias.  sv/sh slots are dead here.
                    X2 = pool.tile([P, HW], f32, tag="sv", name=f"x2{t}")
                    Y2 = pool.tile([P, HW], f32, tag="sh", name=f"y2{t}")
                    nc.gpsimd.tensor_tensor(X2[:], GX[:], GX[:], Alu.mult)
                    nc.gpsimd.tensor_tensor(Y2[:], GY[:], GY[:], Alu.mult)
                    G2 = pool.tile([P, HW], f32, tag="g2", name=f"g2{t}")
                    # eps dropped: sqrt(0)=0 -> zero tent weight, harmless
                    nc.gpsimd.tensor_tensor(G2[:], X2[:], Y2[:], Alu.add)
                    # mag feeds only the tent weights; plain ACT sqrt is ample
                    nc.scalar.activation(MW[:, s, :], G2[:], Act.Sqrt)

                    # gxs = gx + 1e-18, clamped away from 0 (sign-preserving)
                    # so the reciprocal seed never sees 0/denormal. In-place
                    # over GX (forward stream, read precedes write).
                    nc.vector._custom_dve(CLAMPADD, out=GX[:], in0=GX[:],
                                          s0=1e-18, s1=1e-30)
                    RC = pool.tile([P, HW], f32, tag=f"rc{s}", bufs=1,
                                   name=f"rc{t}")
                    nc.vector.reciprocal_approx_fast(RC[:], GX[:])
                    slot[t] = (GX, RC, GY)

                # ---- phase B: orientation (ACT arctan table) ----
                for t in tiles:
                    s = t % GROUP
                    GXS, RC, GY = slot[t]
                    U = pool.tile([P, HW], f32, tag="sv", name=f"u{t}")
                    nc.gpsimd.tensor_tensor(U[:], GY[:], RC[:], Alu.mult)
                    AU = pool.tile([P, HW], f32, tag="sh", name=f"au{t}")
                    nc.scalar.activation(AU[:], U[:], Act.Arctan)
                    # fused quadrant+add: t = au + (gxs<0 ? +/-pi : 0)
                    nc.vector._custom_dve(QUADADD, out=AW[:, s, :],
                                          in0=AU[:], in1=GXS[:], s0=PI)

                # ---- histogram: one scan op per bin over the supergroup ----
                CB = pool.tile([P, GROUP + 1, NBINS + 2], f32, tag="cb",
                               name=f"cb{g}")
                nc.gpsimd.memset(CB[:, 0:1, :], 0.0)
                for k in range(NBINS):
                    o = (CB[:, 1:GROUP + 1, k + 1:k + 2]
                         .broadcast_to([P, GROUP, HW]))
                    if k == 0:
                        nc.vector._custom_dve(
                            WRAP_SCAN, out=o, in0=AW[:], in1=MW[:],
                            s0=-17.0 * D, s1=17.0 * D)
                    else:
                        nc.vector._custom_dve(
                            TRI_SCAN, out=o, in0=AW[:], in1=MW[:],
                            s0=c_lo(k), s1=c_hi(k))
                # per-patch-column bins = adjacent diffs of the cumsums
                nc.vector.tensor_tensor(
                    HEXT[:, g * GROUP:(g + 1) * GROUP, 1:NBINS + 1],
                    CB[:, 1:GROUP + 1, 1:NBINS + 1],
                    CB[:, 0:GROUP, 1:NBINS + 1], Alu.subtract)

            # ---- tail: smoothing, argmax, refinement (batched) ----
            nc.vector.tensor_copy(HEXT[:, :, 0:1], HEXT[:, :, 36:37])
            nc.vector.tensor_copy(HEXT[:, :, 37:38], HEXT[:, :, 1:2])

            SM = pp.tile([P, n_tiles, NBINS], f32)
            nc.vector.tensor_scalar(SM[:], HEXT[:, :, 2:38], w2, None,
                                    Alu.mult)
            nc.vector.scalar_tensor_tensor(
                out=SM[:], in0=HEXT[:, :, 0:36], scalar=w0, in1=SM[:],
                op0=Alu.mult, op1=Alu.add)
            HS = pp.tile([P, n_tiles, NBINS], f32)
            nc.vector.scalar_tensor_tensor(
                out=HS[:], in0=HEXT[:, :, 1:37], scalar=w1, in1=SM[:],
                op0=Alu.mult, op1=Alu.add)

            VMAX = pp.tile([P, n_tiles, 1], f32)
            nc.vector.tensor_reduce(VMAX[:], HS[:], mybir.AxisListType.X,
                                    Alu.max)
            EQ = pp.tile([P, n_tiles, NBINS], f32)
            hs_b, vmax_b = broadcast_tensor_aps(HS[:], VMAX[:])
            nc.vector.tensor_tensor(EQ[:], hs_b, vmax_b, Alu.is_equal)
            nc.vector.tensor_tensor(EQ[:], EQ[:], IOTA64[:], Alu.mult)
            IDX = pp.tile([P, n_tiles, 1], f32)
            nc.vector.tensor_reduce(IDX[:], EQ[:], mybir.AxisListType.X,
                                    Alu.min)
            nc.vector.tensor_scalar(IDX[:], IDX[:], 64.0, None, Alu.add)

            def neighbor_value(shift, wrap_thr, wrap_add, nm):
                IDXN = pp.tile([P, n_tiles, 1], f32, name=f"idxn_{nm}")
                nc.vector.tensor_scalar(IDXN[:], IDX[:], float(shift), None,
                                        Alu.add)
                WADJ = pp.tile([P, n_tiles, 1], f32, name=f"wadj_{nm}")
                if wrap_add < 0:
                    nc.vector.tensor_scalar(WADJ[:], IDXN[:], wrap_thr,
                                            float(wrap_add), Alu.is_gt,
                                            Alu.mult)
                else:
                    nc.vector.tensor_scalar(WADJ[:], IDXN[:], wrap_thr,
                                            float(wrap_add), Alu.is_lt,
                                            Alu.mult)
                nc.vector.tensor_tensor(IDXN[:], IDXN[:], WADJ[:], Alu.add)
                DIF = pp.tile([P, n_tiles, NBINS], f32, name=f"dif_{nm}")
                iota_b, idxn_b = broadcast_tensor_aps(IOTA[:], IDXN[:])
                nc.vector.tensor_tensor(DIF[:], iota_b, idxn_b, Alu.subtract)
                nc.vector.tensor_scalar(DIF[:], DIF[:], 0.0, None,
                                        Alu.is_equal)
                nc.vector.tensor_tensor(DIF[:], DIF[:], HS[:], Alu.mult)
                V = pp.tile([P, n_tiles, 1], f32, name=f"v_{nm}")
                nc.vector.tensor_reduce(V[:], DIF[:], mybir.AxisListType.X,
                                        Alu.add)
                return V

            VP = neighbor_value(+1, 35.5, -36.0, "p")
            VM = neighbor_value(-1, -0.5, +36.0, "m")

            NUM = pp.tile([P, n_tiles, 1], f32)
            nc.vector.tensor_tensor(NUM[:], VP[:], VM[:], Alu.subtract)
            SUMN = pp.tile([P, n_tiles, 1], f32)
            nc.vector.tensor_tensor(SUMN[:], VP[:], VM[:], Alu.add)
            DEN = pp.tile([P, n_tiles, 1], f32)
            nc.vector.tensor_scalar(DEN[:], VMAX[:], 2.0, None, Alu.mult)
            nc.vector.tensor_tensor(DEN[:], DEN[:], SUMN[:], Alu.subtract)
            RECD = pp.tile([P, n_tiles, 1], f32)
            SCD = pp.tile([P, n_tiles, 1], f32)
            nc.vector.reciprocal_approx_accurate(RECD[:], DEN[:], SCD[:])
            REF = pp.tile([P, n_tiles, 1], f32)
            nc.vector.scalar_tensor_tensor(
                out=REF[:], in0=NUM[:], scalar=0.5, in1=RECD[:],
                op0=Alu.mult, op1=Alu.mult)
            nc.vector.tensor_tensor(REF[:], IDX[:], REF[:], Alu.add)
            nc.vector.tensor_scalar(ANG[:], REF[:, :, 0], -2.0 * PI / NBINS,
                                    PI, Alu.mult, Alu.add)

            out_view = out_t[:].rearrange("(t p) -> p t", p=P)
            nc.sync.dma_start(out_view, ANG[:])

    nc.compile()
    return nc


def _get_built(b_core, smooth_w, wk_is_ones):
    key = (b_core, tuple(float(x) for x in smooth_w), bool(wk_is_ones))
    if key not in _BUILD_CACHE:
        _BUILD_CACHE[key] = _build(b_core, smooth_w, wk_is_ones)
    return _BUILD_CACHE[key]


# --------------------------------------------------------------------------
# host entry point
# --------------------------------------------------------------------------
def kernel(patch, weight_kernel, smooth_w):
    from concourse import bass_utils

    patch = np.ascontiguousarray(np.asarray(patch, dtype=np.float32))
    weight_kernel = np.asarray(weight_kernel, dtype=np.float32)
    smooth_w = np.asarray(smooth_w, dtype=np.float32)

    B = patch.shape[0]
    assert B % (N_CORES * P) == 0, f"B={B} not divisible by {N_CORES * P}"
    b_core = B // N_CORES
    n_tiles = b_core // P

    wk_is_ones = bool(np.all(weight_kernel == 1.0))
    nc = _get_built(b_core, smooth_w, wk_is_ones)

    x = patch.reshape(N_CORES, b_core, HW)

    iota = np.tile(np.arange(NBINS, dtype=np.float32), n_tiles)
    consts_row = np.concatenate([iota, iota - 64.0]).astype(np.float32)
    consts = np.ascontiguousarray(
        np.broadcast_to(consts_row, (P, consts_row.size)))

    in_maps = []
    for i in range(N_CORES):
        m = {"patch": np.ascontiguousarray(x[i]), "consts": consts}
        if not wk_is_ones:
            m["wk"] = np.ascontiguousarray(
                np.broadcast_to(weight_kernel.reshape(-1), (P, HW)))
        in_maps.append(m)

    res = bass_utils.run_bass_kernel_spmd(nc, in_maps,
                                          core_ids=list(range(N_CORES)))
    out = np.concatenate([r["angle"] for r in res.results])
    return out.astype(np.float32)
